# revision 21
# baseline (speedup 1.0000x reference)
"""Cross-attention Trainium2 kernel (B=8, N=2048, C=768, head=1).

reference:
  q = q_x @ Wq.T ; k = k_x @ Wk.T
  S = (q @ k.T) / 768 ; P = softmax(S, -1) ; out = P @ v_x

Strategy (per core, data-parallel over batch), fp8 DoubleRow everywhere:
  M16 = 16 * Wq.T @ Wk  (host, fp8)
  qT/kT via PE transposes (bf16 in, fp8 out via the psum drain copy)
  t16T[c2, n] = sum_c1 M16[c1,c2] * qT[c1,n]     (fp8 DR, psum f32 -> fp8)
  S16[m, n]  = sum_c2 kT[c2,m] * t16T[c2,n]      (fp8 DR)  == 16 * S_true
  PT16 = exp(S16/(768*16) + ln16) = 16*exp(a)    (ACT, bf16)
  E16  = PT16 - 16 = 16*(exp(a)-1)               (DVE, fp8; |E16| ~ 0.6)
  O[n, 0:784] = sum_m E16[m,n] * vb8[m, 0:784]   (fp8 DR; col 768 = 16*(Z-N))
  out[n, c] = (O[n,c] + 16*colsum(v)[c]) / (O[n,768] + 16*N)
    colsum from bf16 v tiles on the PE (all-16s matmul, identical rows);
    the += colsum and += 16*N happen on DVE during the psum drain, the
    *1/Z on ACT. 16s cancel in the ratio.

Mean-subtraction is load-bearing: attention here is near-uniform
(a ~ N(0, 0.036)), so out ~= colmean(v) and coherent fp8 noise on P or v
does not average down (naive fp8 P@v measures 3.6e-2 rel err, over the
2e-2 gate). Quantizing only the deviation E = P-1 scales that noise by
~0.036: measured 3.2e-3 end to end in numpy.

Engine split: PE = transposes + t/S/PV DR matmuls + colsum. ACT = exp,
csum psum drain, final normalize-scale. DVE = transpose/t psum drains
(the fp8 casts), E16, the +colsum adds and reciprocals. GpSimd idles
(its tensor_copy is ~6x slower than DVE; measured, do not use). All
DMA is plain loads/stores: XBAR DMA-transpose measured ~25x worse
DMA-time per byte and starved the prologue (and its multi-job sub-
transfer completion racing plain loads on the same ring produced
nondeterministic corruption). PE order S(nb+1) before PV(nb) keeps the
exp/E16 latency off the PE critical path.
"""

import sys

sys.path.insert(0, "/opt/trn_rl_repo")

from contextlib import ExitStack

import numpy as np

import concourse.bass as bass
import concourse.mybir as mybir
import concourse.tile as tile
from concourse import bacc
from concourse.masks import make_identity

F32 = mybir.dt.float32
F32R = mybir.dt.float32r
BF16 = mybir.dt.bfloat16
F8 = mybir.dt.float8e4
DR = mybir.MatmulPerfMode.DoubleRow

B = 8
N = 2048
C = 768
P = 128
CC = C // P          # 6 chunks of the channel dim
NN = N // P          # 16 chunks of the sequence dim
BLK = 512            # free-dim block (PSUM bank = 512 f32)
NB = N // BLK        # 4 sequence blocks
VW = C + 16          # padded v width: [v | 1 | 0*15], stride mult of 16
SCALE16 = 1.0 / float(C * 16)
LN16 = float(np.log(16.0))
Z0 = 16.0 * N        # denominator offset: 16*2048
EXP = mybir.ActivationFunctionType.Exp
COPY = mybir.ActivationFunctionType.Copy
SUB = mybir.AluOpType.subtract
ADD = mybir.AluOpType.add


def build_kernel():
    nc = bacc.Bacc("TRN2", target_bir_lowering=False, debug=False, num_devices=B)
    q_x = nc.declare_dram_parameter("q_x", [N, C], BF16, isOutput=False)
    k_x = nc.declare_dram_parameter("k_x", [N, C], BF16, isOutput=False)
    v_x = nc.declare_dram_parameter("v_x", [N, C], BF16, isOutput=False)
    Mw = nc.declare_dram_parameter("Mw", [C, C], F8, isOutput=False)
    out = nc.declare_dram_parameter("out", [N, C], BF16, isOutput=True)

    with tile.TileContext(nc) as tc, ExitStack() as ctx:
        persist = ctx.enter_context(tc.tile_pool(name="persist", bufs=1))
        qT8 = persist.tile([P, CC, N], F8)      # q_x.T fp8 [c1, n]
        kT8 = persist.tile([P, CC, N], F8)      # k_x.T fp8 [c2, m]
        tT8 = persist.tile([P, CC, N], F8)      # t16.T fp8 [c2, n]
        sbM8 = persist.tile([P, CC, C], F8)     # M16 [c1, c2]
        vb8 = persist.tile([P, NN, VW], F8)     # [v | 1 | 0...] fp8
        csum = persist.tile([P, C], F32)        # 16*colsum(v), identical rows
        identf = persist.tile([P, P], F32)
        identb = persist.tile([P, P], BF16)
        all16 = persist.tile([P, P], BF16)
        ln16 = persist.tile([P, 1], F32)
        onepad = persist.tile([P, NN, VW - C], F32)
        make_identity(nc, identf)
        nc.vector.tensor_copy(out=identb, in_=identf)
        nc.vector.memset(all16, 16.0)
        nc.vector.memset(ln16, LN16)
        nc.vector.memset(onepad[:, :, 0:1], 1.0)
        nc.vector.memset(onepad[:, :, 1:], 0.0)
        nc.vector.tensor_copy(out=vb8[:, :, C:VW], in_=onepad)

        bfpool = ctx.enter_context(tc.tile_pool(name="bfstage", bufs=1))
        qstage = [bfpool.tile([P, C], BF16, name=f"q{mc}") for mc in range(NN)]
        kstage = [bfpool.tile([P, C], BF16, name=f"k{mc}") for mc in range(NN)]
        vstage = [bfpool.tile([P, C], BF16, name=f"v{mc}") for mc in range(NN)]

        # ---------------- prologue DMAs (plain loads only) ----------------
        # sync ring: q block 0, M, rest of q. scalar ring: k. gpsimd: v.
        for mc in range(4):
            nc.sync.dma_start(out=qstage[mc], in_=q_x[mc * P : (mc + 1) * P, :])
        for c1c in range(CC):
            nc.sync.dma_start(
                out=sbM8[:, c1c, :], in_=Mw[c1c * P : (c1c + 1) * P, :]
            )
        for mc in range(4, NN):
            nc.sync.dma_start(out=qstage[mc], in_=q_x[mc * P : (mc + 1) * P, :])
        for mc in range(NN):
            nc.scalar.dma_start(out=kstage[mc], in_=k_x[mc * P : (mc + 1) * P, :])
        for mc in range(NN):
            nc.gpsimd.dma_start(out=vstage[mc], in_=v_x[mc * P : (mc + 1) * P, :])

        # ---------------- PE warmup (HAM un-throttle) ----------------
        with (
            tc.tile_pool(name="warm", bufs=1) as warm_pool,
            tc.tile_pool(name="warm_psum", bufs=1, space="PSUM") as warm_psum,
        ):
            wl = warm_pool.tile([P, P], BF16)
            wr = warm_pool.tile([P, BLK], BF16)
            nc.vector.memset(wl, 0.0)
            nc.vector.memset(wr, 0.0)
            wps = warm_psum.tile([P, BLK], F32)
            for i in range(10):
                nc.tensor.matmul(wps, wl, wr, start=True, stop=True)

        # ---------------- transpose + t phase ----------------
        t_psum = None

        def tr_group(nb, stage, dst, tag, pool=None, ptag="tr"):
                # PE-transpose 4 seq-tiles x 6 chunks into dst[:, cc, nb*BLK..]
                pool = pool or t_psum
                sl = slice(nb * BLK, (nb + 1) * BLK)
                for cc in range(CC):
                    trp = pool.tile(
                        [P, BLK], BF16, tag=ptag, name=f"{tag}{nb}_{cc}"
                    )
                    for j in range(4):
                        nc.tensor.transpose(
                            trp[:, j * P : (j + 1) * P],
                            stage[4 * nb + j][:, cc * P : (cc + 1) * P],
                            identb,
                        )
                    nc.vector.tensor_copy(out=dst[:, cc, sl], in_=trp)

        with tc.tile_pool(name="t_psum", bufs=4, space="PSUM") as t_psum:
            for nb in range(NB):
                sl = slice(nb * BLK, (nb + 1) * BLK)
                tr_group(nb, qstage, qT8, "trq")
                if nb == 0:
                    tr_group(0, kstage, kT8, "trk")
                for c2c in range(CC):
                    tps = t_psum.tile([P, BLK], F32, tag="tp", name=f"t{nb}_{c2c}")
                    for i in range(CC // 2):
                        nc.tensor.matmul(
                            tps,
                            sbM8[:, 2 * i : 2 * i + 2, c2c * P : (c2c + 1) * P],
                            qT8[:, 2 * i : 2 * i + 2, sl],
                            start=(i == 0),
                            stop=(i == CC // 2 - 1),
                            perf_mode=DR,
                        )
                    nc.vector.tensor_copy(out=tT8[:, c2c, sl], in_=tps)
            for mc in range(NN):
                nc.vector.tensor_copy(out=vb8[:, mc, 0:C], in_=vstage[mc])

        # ---------------- steady ----------------
        with (
            tc.tile_pool(name="pt_pool", bufs=1) as pt_pool,
            tc.tile_pool(name="e8_pool", bufs=2) as e8_pool,
            tc.tile_pool(name="out_pool", bufs=2) as out_pool,
            tc.tile_pool(name="sum_pool", bufs=2) as sum_pool,
            tc.tile_pool(name="rec_pool", bufs=2) as rec_pool,
            tc.tile_pool(name="s_psum", bufs=3, space="PSUM") as s_psum,
            tc.tile_pool(name="o_psum", bufs=2, space="PSUM") as o_psum,
            tc.tile_pool(name="o2_psum", bufs=2, space="PSUM") as o2_psum,
        ):
            PT16 = pt_pool.tile([P, NN, BLK], BF16)
            E8s = [
                e8_pool.tile([P, NN, BLK], F8, tag="e8", name=f"e8_{par}")
                for par in range(2)
            ]

            def s_block(nb, weave_trk=False):
                E8 = E8s[nb % 2]
                sl = slice(nb * BLK, (nb + 1) * BLK)
                for mc in range(NN):
                    if weave_trk and mc % 4 == 0 and mc // 4 < NB - 1:
                        tr_group(mc // 4 + 1, kstage, kT8, "trk", pool=s_psum, ptag="sp")
                    sp = s_psum.tile([P, BLK], F32, tag="sp", name=f"sp{nb}_{mc}")
                    for i in range(CC // 2):
                        nc.tensor.matmul(
                            sp,
                            kT8[:, 2 * i : 2 * i + 2, mc * P : (mc + 1) * P],
                            tT8[:, 2 * i : 2 * i + 2, sl],
                            start=(i == 0),
                            stop=(i == CC // 2 - 1),
                            perf_mode=DR,
                        )
                    nc.scalar.activation(
                        out=PT16[:, mc, :], in_=sp, func=EXP,
                        scale=SCALE16, bias=ln16,
                    )
                    nc.vector.tensor_scalar(
                        out=E8[:, mc, :], in0=PT16[:, mc, :],
                        scalar1=16.0, scalar2=None, op0=SUB,
                    )

            def colsum_block():
                # 16*colsum(v) into idle PV psum slots; identical rows
                cs1 = o_psum.tile([P, BLK], F32, tag="op1", name="cs1")
                cs2 = o2_psum.tile([P, C - BLK], F32, tag="op2", name="cs2")
                for mc in range(NN):
                    nc.tensor.matmul(
                        cs1, all16, vstage[mc][:, 0:BLK],
                        start=(mc == 0), stop=(mc == NN - 1),
                    )
                    nc.tensor.matmul(
                        cs2, all16, vstage[mc][:, BLK:C],
                        start=(mc == 0), stop=(mc == NN - 1),
                    )
                nc.scalar.activation(out=csum[:, 0:BLK], in_=cs1, func=COPY)
                nc.scalar.activation(out=csum[:, BLK:C], in_=cs2, func=COPY)

            def pv_block(nb):
                E8 = E8s[nb % 2]
                for ns in range(4):
                    op1 = o_psum.tile([P, BLK], F32, tag="op1", name=f"o1_{nb}_{ns}")
                    op2 = o2_psum.tile(
                        [P, VW - BLK], F32, tag="op2", name=f"o2_{nb}_{ns}"
                    )
                    nsl = slice(ns * P, (ns + 1) * P)
                    for i in range(NN // 2):
                        lhs = E8[:, 2 * i : 2 * i + 2, nsl]
                        first = i == 0
                        last = i == NN // 2 - 1
                        nc.tensor.matmul(
                            op1, lhs, vb8[:, 2 * i : 2 * i + 2, 0:BLK],
                            start=first, stop=last,
                            perf_mode=DR,
                        )
                        nc.tensor.matmul(
                            op2, lhs, vb8[:, 2 * i : 2 * i + 2, BLK:VW],
                            start=first, stop=last,
                            perf_mode=DR,
                        )
                    # out = (O + 16*colsum) / (O_768 + 16*N): adds on DVE,
                    # reciprocal-scale on ACT, 16s cancel
                    zf = rec_pool.tile([P, 1], F32, tag="zf", name=f"zf{nb}_{ns}")
                    rec = rec_pool.tile([P, 1], F32, tag="rec", name=f"rc{nb}_{ns}")
                    nc.vector.tensor_scalar(
                        out=zf, in0=op2[:, C - BLK : C - BLK + 1],
                        scalar1=Z0, scalar2=None, op0=ADD,
                    )
                    nc.vector.reciprocal(out=rec, in_=zf)
                    t12 = sum_pool.tile(
                        [P, C], F32, tag="t12", name=f"t12_{nb}_{ns}"
                    )
                    nc.vector.tensor_tensor(
                        out=t12[:, 0:BLK], in0=op1, in1=csum[:, 0:BLK], op=ADD
                    )
                    nc.vector.tensor_tensor(
                        out=t12[:, BLK:C], in0=op2[:, 0 : C - BLK],
                        in1=csum[:, BLK:C], op=ADD,
                    )
                    o_t = out_pool.tile([P, C], BF16, tag="ot", name=f"ot{nb}_{ns}")
                    nc.scalar.activation(
                        out=o_t[:, 0:BLK], in_=t12[:, 0:BLK], func=COPY, scale=rec
                    )
                    nc.scalar.activation(
                        out=o_t[:, BLK:C], in_=t12[:, BLK:C], func=COPY, scale=rec
                    )
                    row0 = nb * BLK + ns * P
                    if nb == NB - 1:
                        nc.sync.dma_start(
                            out=out[row0 : row0 + P, 0 : C // 2],
                            in_=o_t[:, 0 : C // 2],
                        )
                        nc.scalar.dma_start(
                            out=out[row0 : row0 + P, C // 2 : C],
                            in_=o_t[:, C // 2 : C],
                        )
                    else:
                        ring = nc.sync if ns % 2 == 0 else nc.gpsimd
                        ring.dma_start(out=out[row0 : row0 + P, :], in_=o_t)

            colsum_block()
            s_block(0, weave_trk=True)
            s_block(1)
            pv_block(0)
            s_block(2)
            pv_block(1)
            s_block(3)
            pv_block(2)
            pv_block(3)

    nc.compile()
    return nc


_NC = None


def _get_nc():
    global _NC
    if _NC is None:
        _NC = build_kernel()
    return _NC


def kernel(q_x, k_x, v_x, Wq, Wk):
    import ml_dtypes
    from concourse.bass_utils import run_bass_kernel_spmd

    bf = ml_dtypes.bfloat16
    f8 = ml_dtypes.float8_e4m3
    q_x = np.ascontiguousarray(np.asarray(q_x, dtype=np.float32)).astype(bf)
    k_x = np.ascontiguousarray(np.asarray(k_x, dtype=np.float32)).astype(bf)
    v_x = np.ascontiguousarray(np.asarray(v_x, dtype=np.float32)).astype(bf)
    Wq = np.asarray(Wq, dtype=np.float32)
    Wk = np.asarray(Wk, dtype=np.float32)
    # weight folding: S = q_x (Wq^T Wk) k_x^T; x16 to center fp8 range
    Mw = np.ascontiguousarray(16.0 * (Wq.T @ Wk)).astype(f8)

    nc = _get_nc()
    in_maps = [
        {"q_x": q_x[i], "k_x": k_x[i], "v_x": v_x[i], "Mw": Mw}
        for i in range(B)
    ]
    res = run_bass_kernel_spmd(nc, in_maps, core_ids=list(range(B)))
    return np.stack(
        [res.results[i]["out"].astype(np.float32) for i in range(B)], axis=0
    )


# revision 22
# speedup vs baseline: 1.1475x; 1.1475x over previous
"""Cross-attention Trainium2 kernel (B=8, N=2048, C=768, head=1).

reference:
  q = q_x @ Wq.T ; k = k_x @ Wk.T
  S = (q @ k.T) / 768 ; P = softmax(S, -1) ; out = P @ v_x

Strategy (per core, data-parallel over batch), fp8 DoubleRow everywhere:
  M16 = 16 * Wq.T @ Wk  (host, fp8)
  qT/kT via PE transposes (bf16 in, fp8 out via the psum drain copy)
  t16T[c2, n] = sum_c1 M16[c1,c2] * qT[c1,n]     (fp8 DR, psum f32 -> fp8)
  S16[m, n]  = sum_c2 kT[c2,m] * t16T[c2,n]      (fp8 DR)  == 16 * S_true
  PT16 = exp(S16/(768*16) + ln16) = 16*exp(a)    (ACT, bf16)
  E16  = PT16 - 16 = 16*(exp(a)-1)               (DVE, fp8; |E16| ~ 0.6)
  O[n, 0:784] = sum_m E16[m,n] * vb8[m, 0:784]   (fp8 DR; col 768 = 16*(Z-N))
  out[n, c] = (O[n,c] + 16*colsum(v)[c]) / (O[n,768] + 16*N)
    colsum from bf16 v tiles on the PE (all-16s matmul, identical rows);
    the += colsum and += 16*N happen on DVE during the psum drain, the
    *1/Z on ACT. 16s cancel in the ratio.

Mean-subtraction is load-bearing: attention here is near-uniform
(a ~ N(0, 0.036)), so out ~= colmean(v) and coherent fp8 noise on P or v
does not average down (naive fp8 P@v measures 3.6e-2 rel err, over the
2e-2 gate). Quantizing only the deviation E = P-1 scales that noise by
~0.036: measured 3.2e-3 end to end in numpy.

Engine split: PE = transposes + t/S/PV DR matmuls + colsum. ACT = exp,
csum psum drain, final normalize-scale. DVE = transpose/t psum drains
(the fp8 casts), E16, the +colsum adds and reciprocals. GpSimd idles
(its tensor_copy is ~6x slower than DVE; measured, do not use). All
DMA is plain loads/stores: XBAR DMA-transpose measured ~25x worse
DMA-time per byte and starved the prologue (and its multi-job sub-
transfer completion racing plain loads on the same ring produced
nondeterministic corruption). PE order S(nb+1) before PV(nb) keeps the
exp/E16 latency off the PE critical path.
"""

import sys

sys.path.insert(0, "/opt/trn_rl_repo")

from contextlib import ExitStack

import numpy as np

import concourse.bass as bass
import concourse.mybir as mybir
import concourse.tile as tile
from concourse import bacc
from concourse.masks import make_identity

F32 = mybir.dt.float32
F32R = mybir.dt.float32r
BF16 = mybir.dt.bfloat16
F8 = mybir.dt.float8e4
DR = mybir.MatmulPerfMode.DoubleRow

B = 8
N = 2048
C = 768
P = 128
CC = C // P          # 6 chunks of the channel dim
NN = N // P          # 16 chunks of the sequence dim
BLK = 512            # free-dim block (PSUM bank = 512 f32)
NB = N // BLK        # 4 sequence blocks
VW = C + 16          # padded v width: [v | 1 | 0*15], stride mult of 16
SCALE16 = 1.0 / float(C * 16)
LN16 = float(np.log(16.0))
Z0 = 16.0 * N        # denominator offset: 16*2048
EXP = mybir.ActivationFunctionType.Exp
COPY = mybir.ActivationFunctionType.Copy
SUB = mybir.AluOpType.subtract
ADD = mybir.AluOpType.add


def build_kernel():
    nc = bacc.Bacc("TRN2", target_bir_lowering=False, debug=False, num_devices=B)
    q_x = nc.declare_dram_parameter("q_x", [N, C], BF16, isOutput=False)
    k_x = nc.declare_dram_parameter("k_x", [N, C], BF16, isOutput=False)
    v_x = nc.declare_dram_parameter("v_x", [N, C], BF16, isOutput=False)
    Mw = nc.declare_dram_parameter("Mw", [C, C], F8, isOutput=False)
    out = nc.declare_dram_parameter("out", [N, C], BF16, isOutput=True)

    with tile.TileContext(nc) as tc, ExitStack() as ctx:
        persist = ctx.enter_context(tc.tile_pool(name="persist", bufs=1))
        qT8 = persist.tile([P, CC, N], F8)      # q_x.T fp8 [c1, n]
        kT8 = persist.tile([P, CC, N], F8)      # k_x.T fp8 [c2, m]
        tT8 = persist.tile([P, CC, N], F8)      # t16.T fp8 [c2, n]
        sbM8 = persist.tile([P, CC, C], F8)     # M16 [c1, c2]
        vb8 = persist.tile([P, NN, VW], F8)     # [v | 1 | 0...] fp8
        csum = persist.tile([P, C], F32)        # 16*colsum(v), identical rows
        identf = persist.tile([P, P], F32)
        identb = persist.tile([P, P], BF16)
        all16 = persist.tile([P, P], BF16)
        ln16 = persist.tile([P, 1], F32)
        onepad = persist.tile([P, NN, VW - C], F32)
        make_identity(nc, identf)
        nc.vector.tensor_copy(out=identb, in_=identf)
        nc.vector.memset(all16, 16.0)
        nc.vector.memset(ln16, LN16)
        nc.vector.memset(onepad[:, :, 0:1], 1.0)
        nc.vector.memset(onepad[:, :, 1:], 0.0)
        nc.vector.tensor_copy(out=vb8[:, :, C:VW], in_=onepad)

        bfpool = ctx.enter_context(tc.tile_pool(name="bfstage", bufs=1))
        qstage = [bfpool.tile([P, C], BF16, name=f"q{mc}") for mc in range(NN)]
        kstage = [bfpool.tile([P, C], BF16, name=f"k{mc}") for mc in range(NN)]
        vstage = [bfpool.tile([P, C], BF16, name=f"v{mc}") for mc in range(NN)]

        # ---------------- prologue DMAs (plain loads only) ----------------
        # sync ring: q block 0, M, rest of q. scalar ring: k. gpsimd: v.
        for mc in range(4):
            nc.sync.dma_start(out=qstage[mc], in_=q_x[mc * P : (mc + 1) * P, :])
        for c1c in range(CC):
            nc.sync.dma_start(
                out=sbM8[:, c1c, :], in_=Mw[c1c * P : (c1c + 1) * P, :]
            )
        for mc in range(4, NN):
            nc.sync.dma_start(out=qstage[mc], in_=q_x[mc * P : (mc + 1) * P, :])
        for mc in range(NN):
            nc.scalar.dma_start(out=kstage[mc], in_=k_x[mc * P : (mc + 1) * P, :])
        for mc in range(NN):
            nc.gpsimd.dma_start(out=vstage[mc], in_=v_x[mc * P : (mc + 1) * P, :])

        # ---------------- PE warmup (HAM un-throttle) ----------------
        with (
            tc.tile_pool(name="warm", bufs=1) as warm_pool,
            tc.tile_pool(name="warm_psum", bufs=1, space="PSUM") as warm_psum,
        ):
            wl = warm_pool.tile([P, P], BF16)
            wr = warm_pool.tile([P, BLK], BF16)
            nc.vector.memset(wl, 0.0)
            nc.vector.memset(wr, 0.0)
            wps = warm_psum.tile([P, BLK], F32)
            for i in range(10):
                nc.tensor.matmul(wps, wl, wr, start=True, stop=True)

        # ---------------- transpose + t phase ----------------
        t_psum = None

        def tr_group(nb, stage, dst, tag, pool=None, ptag="tr"):
                # PE-transpose 4 seq-tiles x 6 chunks into dst[:, cc, nb*BLK..]
                pool = pool or t_psum
                sl = slice(nb * BLK, (nb + 1) * BLK)
                for cc in range(CC):
                    trp = pool.tile(
                        [P, BLK], BF16, tag=ptag, name=f"{tag}{nb}_{cc}"
                    )
                    for j in range(4):
                        nc.tensor.transpose(
                            trp[:, j * P : (j + 1) * P],
                            stage[4 * nb + j][:, cc * P : (cc + 1) * P],
                            identb,
                        )
                    nc.vector.tensor_copy(out=dst[:, cc, sl], in_=trp)

        def tr_chunk(nb, cc, stage, dst, tag):
            sl = slice(nb * BLK, (nb + 1) * BLK)
            trp = t_psum.tile([P, BLK], BF16, tag="tr", name=f"{tag}{nb}_{cc}")
            for j in range(4):
                nc.tensor.transpose(
                    trp[:, j * P : (j + 1) * P],
                    stage[4 * nb + j][:, cc * P : (cc + 1) * P],
                    identb,
                )
            nc.vector.tensor_copy(out=dst[:, cc, sl], in_=trp)

        with tc.tile_pool(name="t_psum", bufs=4, space="PSUM") as t_psum:
            # transposes of block nb+1 (q then k) interleave between the six
            # t(nb) matmul groups: 12 tr-chunks per block, 2 per t-group
            tr_group(0, qstage, qT8, "trq")
            tr_group(0, kstage, kT8, "trk")
            for nb in range(NB):
                sl = slice(nb * BLK, (nb + 1) * BLK)
                for c2c in range(CC):
                    if nb + 1 < NB:
                        nxt = 2 * c2c
                        for x in (nxt, nxt + 1):
                            if x < CC:
                                tr_chunk(nb + 1, x, qstage, qT8, "trq")
                            else:
                                tr_chunk(nb + 1, x - CC, kstage, kT8, "trk")
                    tps = t_psum.tile([P, BLK], F32, tag="tp", name=f"t{nb}_{c2c}")
                    for i in range(CC // 2):
                        nc.tensor.matmul(
                            tps,
                            sbM8[:, 2 * i : 2 * i + 2, c2c * P : (c2c + 1) * P],
                            qT8[:, 2 * i : 2 * i + 2, sl],
                            start=(i == 0),
                            stop=(i == CC // 2 - 1),
                            perf_mode=DR,
                        )
                    nc.vector.tensor_copy(out=tT8[:, c2c, sl], in_=tps)
            for mc in range(NN):
                nc.vector.tensor_copy(out=vb8[:, mc, 0:C], in_=vstage[mc])

        # ---------------- steady ----------------
        with (
            tc.tile_pool(name="pt_pool", bufs=1) as pt_pool,
            tc.tile_pool(name="e8_pool", bufs=2) as e8_pool,
            tc.tile_pool(name="out_pool", bufs=2) as out_pool,
            tc.tile_pool(name="sum_pool", bufs=2) as sum_pool,
            tc.tile_pool(name="rec_pool", bufs=2) as rec_pool,
            tc.tile_pool(name="s_psum", bufs=3, space="PSUM") as s_psum,
            tc.tile_pool(name="o_psum", bufs=2, space="PSUM") as o_psum,
            tc.tile_pool(name="o2_psum", bufs=2, space="PSUM") as o2_psum,
        ):
            PT16 = pt_pool.tile([P, NN, BLK], BF16)
            E8s = [
                e8_pool.tile([P, NN, BLK], F8, tag="e8", name=f"e8_{par}")
                for par in range(2)
            ]

            def s_block(nb):
                E8 = E8s[nb % 2]
                sl = slice(nb * BLK, (nb + 1) * BLK)
                for mc in range(NN):
                    sp = s_psum.tile([P, BLK], F32, tag="sp", name=f"sp{nb}_{mc}")
                    for i in range(CC // 2):
                        nc.tensor.matmul(
                            sp,
                            kT8[:, 2 * i : 2 * i + 2, mc * P : (mc + 1) * P],
                            tT8[:, 2 * i : 2 * i + 2, sl],
                            start=(i == 0),
                            stop=(i == CC // 2 - 1),
                            perf_mode=DR,
                        )
                    nc.scalar.activation(
                        out=PT16[:, mc, :], in_=sp, func=EXP,
                        scale=SCALE16, bias=ln16,
                    )
                    nc.vector.tensor_scalar(
                        out=E8[:, mc, :], in0=PT16[:, mc, :],
                        scalar1=16.0, scalar2=None, op0=SUB,
                    )

            def colsum_block():
                # 16*colsum(v) into idle PV psum slots; identical rows
                cs1 = o_psum.tile([P, BLK], F32, tag="op1", name="cs1")
                cs2 = o2_psum.tile([P, C - BLK], F32, tag="op2", name="cs2")
                for mc in range(NN):
                    nc.tensor.matmul(
                        cs1, all16, vstage[mc][:, 0:BLK],
                        start=(mc == 0), stop=(mc == NN - 1),
                    )
                    nc.tensor.matmul(
                        cs2, all16, vstage[mc][:, BLK:C],
                        start=(mc == 0), stop=(mc == NN - 1),
                    )
                nc.scalar.activation(out=csum[:, 0:BLK], in_=cs1, func=COPY)
                nc.scalar.activation(out=csum[:, BLK:C], in_=cs2, func=COPY)

            def pv_block(nb):
                E8 = E8s[nb % 2]
                for ns in range(4):
                    op1 = o_psum.tile([P, BLK], F32, tag="op1", name=f"o1_{nb}_{ns}")
                    op2 = o2_psum.tile(
                        [P, VW - BLK], F32, tag="op2", name=f"o2_{nb}_{ns}"
                    )
                    nsl = slice(ns * P, (ns + 1) * P)
                    for i in range(NN // 2):
                        lhs = E8[:, 2 * i : 2 * i + 2, nsl]
                        first = i == 0
                        last = i == NN // 2 - 1
                        nc.tensor.matmul(
                            op1, lhs, vb8[:, 2 * i : 2 * i + 2, 0:BLK],
                            start=first, stop=last,
                            perf_mode=DR,
                        )
                        nc.tensor.matmul(
                            op2, lhs, vb8[:, 2 * i : 2 * i + 2, BLK:VW],
                            start=first, stop=last,
                            perf_mode=DR,
                        )
                    # out = (O + 16*colsum) / (O_768 + 16*N): adds on DVE,
                    # reciprocal-scale on ACT, 16s cancel
                    zf = rec_pool.tile([P, 1], F32, tag="zf", name=f"zf{nb}_{ns}")
                    rec = rec_pool.tile([P, 1], F32, tag="rec", name=f"rc{nb}_{ns}")
                    nc.vector.tensor_scalar(
                        out=zf, in0=op2[:, C - BLK : C - BLK + 1],
                        scalar1=Z0, scalar2=None, op0=ADD,
                    )
                    nc.vector.reciprocal(out=rec, in_=zf)
                    t12 = sum_pool.tile(
                        [P, C], F32, tag="t12", name=f"t12_{nb}_{ns}"
                    )
                    nc.vector.tensor_tensor(
                        out=t12[:, 0:BLK], in0=op1, in1=csum[:, 0:BLK], op=ADD
                    )
                    nc.vector.tensor_tensor(
                        out=t12[:, BLK:C], in0=op2[:, 0 : C - BLK],
                        in1=csum[:, BLK:C], op=ADD,
                    )
                    o_t = out_pool.tile([P, C], BF16, tag="ot", name=f"ot{nb}_{ns}")
                    nc.scalar.activation(
                        out=o_t[:, 0:BLK], in_=t12[:, 0:BLK], func=COPY, scale=rec
                    )
                    nc.scalar.activation(
                        out=o_t[:, BLK:C], in_=t12[:, BLK:C], func=COPY, scale=rec
                    )
                    row0 = nb * BLK + ns * P
                    if nb == NB - 1:
                        nc.sync.dma_start(
                            out=out[row0 : row0 + P, 0 : C // 2],
                            in_=o_t[:, 0 : C // 2],
                        )
                        nc.scalar.dma_start(
                            out=out[row0 : row0 + P, C // 2 : C],
                            in_=o_t[:, C // 2 : C],
                        )
                    else:
                        ring = nc.sync if ns % 2 == 0 else nc.gpsimd
                        ring.dma_start(out=out[row0 : row0 + P, :], in_=o_t)

            colsum_block()
            s_block(0)
            s_block(1)
            pv_block(0)
            s_block(2)
            pv_block(1)
            s_block(3)
            pv_block(2)
            pv_block(3)

    nc.compile()
    return nc


_NC = None


def _get_nc():
    global _NC
    if _NC is None:
        _NC = build_kernel()
    return _NC


def kernel(q_x, k_x, v_x, Wq, Wk):
    import ml_dtypes
    from concourse.bass_utils import run_bass_kernel_spmd

    bf = ml_dtypes.bfloat16
    f8 = ml_dtypes.float8_e4m3
    q_x = np.ascontiguousarray(np.asarray(q_x, dtype=np.float32)).astype(bf)
    k_x = np.ascontiguousarray(np.asarray(k_x, dtype=np.float32)).astype(bf)
    v_x = np.ascontiguousarray(np.asarray(v_x, dtype=np.float32)).astype(bf)
    Wq = np.asarray(Wq, dtype=np.float32)
    Wk = np.asarray(Wk, dtype=np.float32)
    # weight folding: S = q_x (Wq^T Wk) k_x^T; x16 to center fp8 range
    Mw = np.ascontiguousarray(16.0 * (Wq.T @ Wk)).astype(f8)

    nc = _get_nc()
    in_maps = [
        {"q_x": q_x[i], "k_x": k_x[i], "v_x": v_x[i], "Mw": Mw}
        for i in range(B)
    ]
    res = run_bass_kernel_spmd(nc, in_maps, core_ids=list(range(B)))
    return np.stack(
        [res.results[i]["out"].astype(np.float32) for i in range(B)], axis=0
    )


# revision 24
# speedup vs baseline: 1.2311x; 1.0729x over previous
"""Cross-attention Trainium2 kernel (B=8, N=2048, C=768, head=1).

reference:
  q = q_x @ Wq.T ; k = k_x @ Wk.T
  S = (q @ k.T) / 768 ; P = softmax(S, -1) ; out = P @ v_x

Strategy (per core, data-parallel over batch), fp8 DoubleRow everywhere:
  M16 = 16 * Wq.T @ Wk  (host, fp8)
  qT/kT via PE transposes (bf16 in, fp8 out via the psum drain copy)
  t16T[c2, n] = sum_c1 M16[c1,c2] * qT[c1,n]     (fp8 DR, psum f32 -> fp8)
  S16[m, n]  = sum_c2 kT[c2,m] * t16T[c2,n]      (fp8 DR)  == 16 * S_true
  PT16 = exp(S16/(768*16) + ln16) = 16*exp(a)    (ACT, bf16)
  E16  = PT16 - 16 = 16*(exp(a)-1)               (DVE, fp8; |E16| ~ 0.6)
  O[n, 0:784] = sum_m E16[m,n] * vb8[m, 0:784]   (fp8 DR; col 768 = 16*(Z-N))
  out[n, c] = (O[n,c] + 16*colsum(v)[c]) / (O[n,768] + 16*N)
    colsum from bf16 v tiles on the PE (all-16s matmul, identical rows);
    the += colsum and += 16*N happen on DVE during the psum drain, the
    *1/Z on ACT. 16s cancel in the ratio.

Mean-subtraction is load-bearing: attention here is near-uniform
(a ~ N(0, 0.036)), so out ~= colmean(v) and coherent fp8 noise on P or v
does not average down (naive fp8 P@v measures 3.6e-2 rel err, over the
2e-2 gate). Quantizing only the deviation E = P-1 scales that noise by
~0.036: measured 3.2e-3 end to end in numpy.

Engine split: PE = transposes + t/S/PV DR matmuls + colsum. ACT = exp,
csum psum drain, final normalize-scale. DVE = transpose/t psum drains
(the fp8 casts), E16, the +colsum adds and reciprocals. GpSimd idles
(its tensor_copy is ~6x slower than DVE; measured, do not use). All
DMA is plain loads/stores: XBAR DMA-transpose measured ~25x worse
DMA-time per byte and starved the prologue (and its multi-job sub-
transfer completion racing plain loads on the same ring produced
nondeterministic corruption). PE order S(nb+1) before PV(nb) keeps the
exp/E16 latency off the PE critical path.
"""

import sys

sys.path.insert(0, "/opt/trn_rl_repo")

from contextlib import ExitStack

import numpy as np

import concourse.bass as bass
import concourse.mybir as mybir
import concourse.tile as tile
from concourse import bacc
from concourse.masks import make_identity

F32 = mybir.dt.float32
F32R = mybir.dt.float32r
BF16 = mybir.dt.bfloat16
F8 = mybir.dt.float8e4
DR = mybir.MatmulPerfMode.DoubleRow

B = 8
N = 2048
C = 768
P = 128
CC = C // P          # 6 chunks of the channel dim
NN = N // P          # 16 chunks of the sequence dim
BLK = 512            # free-dim block (PSUM bank = 512 f32)
NB = N // BLK        # 4 sequence blocks
VW = C + 16          # padded v width: [v | 1 | 0*15], stride mult of 16
SCALE16 = 1.0 / float(C * 16)
LN16 = float(np.log(16.0))
Z0 = 16.0 * N        # denominator offset: 16*2048
EXP = mybir.ActivationFunctionType.Exp
COPY = mybir.ActivationFunctionType.Copy
SUB = mybir.AluOpType.subtract
ADD = mybir.AluOpType.add


def build_kernel():
    nc = bacc.Bacc("TRN2", target_bir_lowering=False, debug=False, num_devices=B)
    q_x = nc.declare_dram_parameter("q_x", [N, C], BF16, isOutput=False)
    k_x = nc.declare_dram_parameter("k_x", [N, C], BF16, isOutput=False)
    v_x = nc.declare_dram_parameter("v_x", [N, C], BF16, isOutput=False)
    Mw = nc.declare_dram_parameter("Mw", [C, C], F8, isOutput=False)
    out = nc.declare_dram_parameter("out", [N, C], BF16, isOutput=True)

    with tile.TileContext(nc) as tc, ExitStack() as ctx:
        persist = ctx.enter_context(tc.tile_pool(name="persist", bufs=1))
        qT8 = persist.tile([P, CC, N], F8)      # q_x.T fp8 [c1, n]
        kT8 = persist.tile([P, CC, N], F8)      # k_x.T fp8 [c2, m]
        tT8 = persist.tile([P, CC, N], F8)      # t16.T fp8 [c2, n]
        sbM8 = persist.tile([P, CC, C], F8)     # M16 [c1, c2]
        vb8 = persist.tile([P, NN, VW], F8)     # [v | 1 | 0...] fp8
        csum = persist.tile([P, C], F32)        # 16*colsum(v), identical rows
        identf = persist.tile([P, P], F32)
        identb = persist.tile([P, P], BF16)
        all16 = persist.tile([P, P], BF16)
        ln16 = persist.tile([P, 1], F32)
        onepad = persist.tile([P, NN, VW - C], F32)

        bfpool = ctx.enter_context(tc.tile_pool(name="bfstage", bufs=1))
        qstage = [bfpool.tile([P, C], BF16, name=f"q{mc}") for mc in range(NN)]
        kstage = [bfpool.tile([P, C], BF16, name=f"k{mc}") for mc in range(NN)]
        vstage = [bfpool.tile([P, C], BF16, name=f"v{mc}") for mc in range(NN)]

        # ---------------- prologue DMAs (plain loads only) ----------------
        # sync ring: q block 0, M, rest of q. scalar ring: k. gpsimd: v.
        for mc in range(4):
            nc.sync.dma_start(out=qstage[mc], in_=q_x[mc * P : (mc + 1) * P, :])
        for c1c in range(CC):
            nc.sync.dma_start(
                out=sbM8[:, c1c, :], in_=Mw[c1c * P : (c1c + 1) * P, :]
            )
        for mc in range(4, NN):
            nc.sync.dma_start(out=qstage[mc], in_=q_x[mc * P : (mc + 1) * P, :])
        for mc in range(NN):
            nc.scalar.dma_start(out=kstage[mc], in_=k_x[mc * P : (mc + 1) * P, :])
        for mc in range(NN):
            nc.gpsimd.dma_start(out=vstage[mc], in_=v_x[mc * P : (mc + 1) * P, :])

        # ---------------- PE warmup (HAM un-throttle) ----------------
        with (
            tc.tile_pool(name="warm", bufs=1) as warm_pool,
            tc.tile_pool(name="warm_psum", bufs=1, space="PSUM") as warm_psum,
        ):
            wl = warm_pool.tile([P, P], BF16)
            wr = warm_pool.tile([P, BLK], BF16)
            nc.vector.memset(wl, 0.0)
            nc.vector.memset(wr, 0.0)
            wps = warm_psum.tile([P, BLK], F32)
            for i in range(10):
                nc.tensor.matmul(wps, wl, wr, start=True, stop=True)

        make_identity(nc, identf)
        nc.vector.tensor_copy(out=identb, in_=identf)
        nc.vector.memset(all16, 16.0)
        nc.vector.memset(ln16, LN16)
        nc.vector.memset(onepad[:, :, 0:1], 1.0)
        nc.vector.memset(onepad[:, :, 1:], 0.0)
        nc.vector.tensor_copy(out=vb8[:, :, C:VW], in_=onepad)

        # ---------------- transpose + t phase ----------------
        t_psum = None

        def tr_group(nb, stage, dst, tag, pool=None, ptag="tr"):
                # PE-transpose 4 seq-tiles x 6 chunks into dst[:, cc, nb*BLK..]
                pool = pool or t_psum
                sl = slice(nb * BLK, (nb + 1) * BLK)
                for cc in range(CC):
                    trp = pool.tile(
                        [P, BLK], BF16, tag=ptag, name=f"{tag}{nb}_{cc}"
                    )
                    for j in range(4):
                        nc.tensor.transpose(
                            trp[:, j * P : (j + 1) * P],
                            stage[4 * nb + j][:, cc * P : (cc + 1) * P],
                            identb,
                        )
                    nc.vector.tensor_copy(out=dst[:, cc, sl], in_=trp)

        def tr_chunk(nb, cc, stage, dst, tag):
            sl = slice(nb * BLK, (nb + 1) * BLK)
            trp = t_psum.tile([P, BLK], BF16, tag="tr", name=f"{tag}{nb}_{cc}")
            for j in range(4):
                nc.tensor.transpose(
                    trp[:, j * P : (j + 1) * P],
                    stage[4 * nb + j][:, cc * P : (cc + 1) * P],
                    identb,
                )
            nc.vector.tensor_copy(out=dst[:, cc, sl], in_=trp)

        with tc.tile_pool(name="t_psum", bufs=4, space="PSUM") as t_psum:
            # transposes of block nb+1 (q then k) interleave between the six
            # t(nb) matmul groups: 12 tr-chunks per block, 2 per t-group
            tr_group(0, qstage, qT8, "trq")
            tr_group(0, kstage, kT8, "trk")
            for nb in range(NB):
                sl = slice(nb * BLK, (nb + 1) * BLK)
                for c2c in range(CC):
                    if nb + 1 < NB:
                        nxt = 2 * c2c
                        for x in (nxt, nxt + 1):
                            if x < CC:
                                tr_chunk(nb + 1, x, qstage, qT8, "trq")
                            else:
                                tr_chunk(nb + 1, x - CC, kstage, kT8, "trk")
                    tps = t_psum.tile([P, BLK], F32, tag="tp", name=f"t{nb}_{c2c}")
                    for i in range(CC // 2):
                        nc.tensor.matmul(
                            tps,
                            sbM8[:, 2 * i : 2 * i + 2, c2c * P : (c2c + 1) * P],
                            qT8[:, 2 * i : 2 * i + 2, sl],
                            start=(i == 0),
                            stop=(i == CC // 2 - 1),
                            perf_mode=DR,
                        )
                    nc.scalar.activation(out=tT8[:, c2c, sl], in_=tps, func=COPY)
            for mc in range(NN):
                nc.vector.tensor_copy(out=vb8[:, mc, 0:C], in_=vstage[mc])

        # ---------------- steady ----------------
        with (
            tc.tile_pool(name="pt_pool", bufs=2) as pt_pool,
            tc.tile_pool(name="e8_pool", bufs=2) as e8_pool,
            tc.tile_pool(name="out_pool", bufs=2) as out_pool,
            tc.tile_pool(name="sum_pool", bufs=2) as sum_pool,
            tc.tile_pool(name="rec_pool", bufs=2) as rec_pool,
            tc.tile_pool(name="s_psum", bufs=3, space="PSUM") as s_psum,
            tc.tile_pool(name="o_psum", bufs=2, space="PSUM") as o_psum,
            tc.tile_pool(name="o2_psum", bufs=2, space="PSUM") as o2_psum,
        ):
            PT16s = [
                pt_pool.tile([P, NN, BLK], BF16, tag="pt", name=f"pt{par}")
                for par in range(2)
            ]
            E8s = [
                e8_pool.tile([P, NN, BLK], F8, tag="e8", name=f"e8_{par}")
                for par in range(2)
            ]

            def s_block(nb):
                E8 = E8s[nb % 2]
                PT16 = PT16s[nb % 2]
                sl = slice(nb * BLK, (nb + 1) * BLK)
                for mc in range(NN):
                    sp = s_psum.tile([P, BLK], F32, tag="sp", name=f"sp{nb}_{mc}")
                    for i in range(CC // 2):
                        nc.tensor.matmul(
                            sp,
                            kT8[:, 2 * i : 2 * i + 2, mc * P : (mc + 1) * P],
                            tT8[:, 2 * i : 2 * i + 2, sl],
                            start=(i == 0),
                            stop=(i == CC // 2 - 1),
                            perf_mode=DR,
                        )
                    nc.scalar.activation(
                        out=PT16[:, mc, :], in_=sp, func=EXP,
                        scale=SCALE16, bias=ln16,
                    )
                    nc.vector.tensor_scalar(
                        out=E8[:, mc, :], in0=PT16[:, mc, :],
                        scalar1=16.0, scalar2=None, op0=SUB,
                    )

            def colsum_block():
                # 16*colsum(v) into idle PV psum slots; identical rows
                cs1 = o_psum.tile([P, BLK], F32, tag="op1", name="cs1")
                cs2 = o2_psum.tile([P, C - BLK], F32, tag="op2", name="cs2")
                for mc in range(NN):
                    nc.tensor.matmul(
                        cs1, all16, vstage[mc][:, 0:BLK],
                        start=(mc == 0), stop=(mc == NN - 1),
                    )
                    nc.tensor.matmul(
                        cs2, all16, vstage[mc][:, BLK:C],
                        start=(mc == 0), stop=(mc == NN - 1),
                    )
                nc.scalar.activation(out=csum[:, 0:BLK], in_=cs1, func=COPY)
                nc.scalar.activation(out=csum[:, BLK:C], in_=cs2, func=COPY)

            def pv_block(nb):
                E8 = E8s[nb % 2]
                for ns in range(4):
                    op1 = o_psum.tile([P, BLK], F32, tag="op1", name=f"o1_{nb}_{ns}")
                    op2 = o2_psum.tile(
                        [P, VW - BLK], F32, tag="op2", name=f"o2_{nb}_{ns}"
                    )
                    nsl = slice(ns * P, (ns + 1) * P)
                    for i in range(NN // 2):
                        lhs = E8[:, 2 * i : 2 * i + 2, nsl]
                        first = i == 0
                        last = i == NN // 2 - 1
                        nc.tensor.matmul(
                            op1, lhs, vb8[:, 2 * i : 2 * i + 2, 0:BLK],
                            start=first, stop=last,
                            perf_mode=DR,
                        )
                        nc.tensor.matmul(
                            op2, lhs, vb8[:, 2 * i : 2 * i + 2, BLK:VW],
                            start=first, stop=last,
                            perf_mode=DR,
                        )
                    # out = (O + 16*colsum) / (O_768 + 16*N): adds on DVE,
                    # reciprocal-scale on ACT, 16s cancel
                    zf = rec_pool.tile([P, 1], F32, tag="zf", name=f"zf{nb}_{ns}")
                    rec = rec_pool.tile([P, 1], F32, tag="rec", name=f"rc{nb}_{ns}")
                    nc.vector.tensor_scalar(
                        out=zf, in0=op2[:, C - BLK : C - BLK + 1],
                        scalar1=Z0, scalar2=None, op0=ADD,
                    )
                    nc.vector.reciprocal(out=rec, in_=zf)
                    t12 = sum_pool.tile(
                        [P, C], F32, tag="t12", name=f"t12_{nb}_{ns}"
                    )
                    nc.vector.tensor_tensor(
                        out=t12[:, 0:BLK], in0=op1, in1=csum[:, 0:BLK], op=ADD
                    )
                    nc.vector.tensor_tensor(
                        out=t12[:, BLK:C], in0=op2[:, 0 : C - BLK],
                        in1=csum[:, BLK:C], op=ADD,
                    )
                    o_t = out_pool.tile([P, C], BF16, tag="ot", name=f"ot{nb}_{ns}")
                    nc.scalar.activation(
                        out=o_t[:, 0:BLK], in_=t12[:, 0:BLK], func=COPY, scale=rec
                    )
                    nc.scalar.activation(
                        out=o_t[:, BLK:C], in_=t12[:, BLK:C], func=COPY, scale=rec
                    )
                    row0 = nb * BLK + ns * P
                    if nb == NB - 1:
                        nc.sync.dma_start(
                            out=out[row0 : row0 + P, 0 : C // 2],
                            in_=o_t[:, 0 : C // 2],
                        )
                        nc.scalar.dma_start(
                            out=out[row0 : row0 + P, C // 2 : C],
                            in_=o_t[:, C // 2 : C],
                        )
                    else:
                        ring = nc.sync if ns % 2 == 0 else nc.gpsimd
                        ring.dma_start(out=out[row0 : row0 + P, :], in_=o_t)

            colsum_block()
            s_block(0)
            s_block(1)
            pv_block(0)
            s_block(2)
            pv_block(1)
            s_block(3)
            pv_block(2)
            pv_block(3)

    nc.compile()
    return nc


_NC = None


def _get_nc():
    global _NC
    if _NC is None:
        _NC = build_kernel()
    return _NC


def kernel(q_x, k_x, v_x, Wq, Wk):
    import ml_dtypes
    from concourse.bass_utils import run_bass_kernel_spmd

    bf = ml_dtypes.bfloat16
    f8 = ml_dtypes.float8_e4m3
    q_x = np.ascontiguousarray(np.asarray(q_x, dtype=np.float32)).astype(bf)
    k_x = np.ascontiguousarray(np.asarray(k_x, dtype=np.float32)).astype(bf)
    v_x = np.ascontiguousarray(np.asarray(v_x, dtype=np.float32)).astype(bf)
    Wq = np.asarray(Wq, dtype=np.float32)
    Wk = np.asarray(Wk, dtype=np.float32)
    # weight folding: S = q_x (Wq^T Wk) k_x^T; x16 to center fp8 range
    Mw = np.ascontiguousarray(16.0 * (Wq.T @ Wk)).astype(f8)

    nc = _get_nc()
    in_maps = [
        {"q_x": q_x[i], "k_x": k_x[i], "v_x": v_x[i], "Mw": Mw}
        for i in range(B)
    ]
    res = run_bass_kernel_spmd(nc, in_maps, core_ids=list(range(B)))
    return np.stack(
        [res.results[i]["out"].astype(np.float32) for i in range(B)], axis=0
    )


# revision 25
# speedup vs baseline: 1.2738x; 1.0347x over previous
"""Cross-attention Trainium2 kernel (B=8, N=2048, C=768, head=1).

reference:
  q = q_x @ Wq.T ; k = k_x @ Wk.T
  S = (q @ k.T) / 768 ; P = softmax(S, -1) ; out = P @ v_x

Strategy (per core, data-parallel over batch), fp8 DoubleRow everywhere:
  M16 = 16 * Wq.T @ Wk  (host, fp8)
  qT/kT via PE transposes (bf16 in, fp8 out via the psum drain copy)
  t16T[c2, n] = sum_c1 M16[c1,c2] * qT[c1,n]     (fp8 DR, psum f32 -> fp8)
  S16[m, n]  = sum_c2 kT[c2,m] * t16T[c2,n]      (fp8 DR)  == 16 * S_true
  PT16 = exp(S16/(768*16) + ln16) = 16*exp(a)    (ACT, bf16)
  E16  = PT16 - 16 = 16*(exp(a)-1)               (DVE, fp8; |E16| ~ 0.6)
  O[n, 0:784] = sum_m E16[m,n] * vb8[m, 0:784]   (fp8 DR; col 768 = 16*(Z-N))
  out[n, c] = (O[n,c] + 16*colsum(v)[c]) / (O[n,768] + 16*N)
    colsum from bf16 v tiles on the PE (all-16s matmul, identical rows);
    the += colsum and += 16*N happen on DVE during the psum drain, the
    *1/Z on ACT. 16s cancel in the ratio.

Mean-subtraction is load-bearing: attention here is near-uniform
(a ~ N(0, 0.036)), so out ~= colmean(v) and coherent fp8 noise on P or v
does not average down (naive fp8 P@v measures 3.6e-2 rel err, over the
2e-2 gate). Quantizing only the deviation E = P-1 scales that noise by
~0.036: measured 3.2e-3 end to end in numpy.

Engine split: PE = transposes + t/S/PV DR matmuls + colsum. ACT = exp,
csum psum drain, final normalize-scale. DVE = transpose/t psum drains
(the fp8 casts), E16, the +colsum adds and reciprocals. GpSimd idles
(its tensor_copy is ~6x slower than DVE; measured, do not use). All
DMA is plain loads/stores: XBAR DMA-transpose measured ~25x worse
DMA-time per byte and starved the prologue (and its multi-job sub-
transfer completion racing plain loads on the same ring produced
nondeterministic corruption). PE order S(nb+1) before PV(nb) keeps the
exp/E16 latency off the PE critical path.
"""

import sys

sys.path.insert(0, "/opt/trn_rl_repo")

from contextlib import ExitStack

import numpy as np

import concourse.bass as bass
import concourse.mybir as mybir
import concourse.tile as tile
from concourse import bacc
from concourse.masks import make_identity

F32 = mybir.dt.float32
F32R = mybir.dt.float32r
BF16 = mybir.dt.bfloat16
F8 = mybir.dt.float8e4
DR = mybir.MatmulPerfMode.DoubleRow

B = 8
N = 2048
C = 768
P = 128
CC = C // P          # 6 chunks of the channel dim
NN = N // P          # 16 chunks of the sequence dim
BLK = 512            # free-dim block (PSUM bank = 512 f32)
NB = N // BLK        # 4 sequence blocks
VW = C + 16          # padded v width: [v | 1 | 0*15], stride mult of 16
SCALE16 = 1.0 / float(C * 16)
LN16 = float(np.log(16.0))
Z0 = 16.0 * N        # denominator offset: 16*2048
EXP = mybir.ActivationFunctionType.Exp
COPY = mybir.ActivationFunctionType.Copy
SUB = mybir.AluOpType.subtract
ADD = mybir.AluOpType.add


def build_kernel():
    nc = bacc.Bacc("TRN2", target_bir_lowering=False, debug=False, num_devices=B)
    q_x = nc.declare_dram_parameter("q_x", [N, C], BF16, isOutput=False)
    k_x = nc.declare_dram_parameter("k_x", [N, C], BF16, isOutput=False)
    v_x = nc.declare_dram_parameter("v_x", [N, C], BF16, isOutput=False)
    Mw = nc.declare_dram_parameter("Mw", [C, C], F8, isOutput=False)
    out = nc.declare_dram_parameter("out", [N, C], BF16, isOutput=True)

    with tile.TileContext(nc) as tc, ExitStack() as ctx:
        persist = ctx.enter_context(tc.tile_pool(name="persist", bufs=1))
        qT8 = persist.tile([P, CC, N], F8)      # q_x.T fp8 [c1, n]
        kT8 = persist.tile([P, CC, N], F8)      # k_x.T fp8 [c2, m]
        tT8 = persist.tile([P, CC, N], F8)      # t16.T fp8 [c2, n]
        sbM8 = persist.tile([P, CC, C], F8)     # M16 [c1, c2]
        vb8 = persist.tile([P, NN, VW], F8)     # [v | 1 | 0...] fp8
        csum = persist.tile([P, C], F32)        # 16*colsum(v), identical rows
        identf = persist.tile([P, P], F32)
        identb = persist.tile([P, P], BF16)
        all16 = persist.tile([P, P], BF16)
        ln16 = persist.tile([P, 1], F32)
        onepad = persist.tile([P, NN, VW - C], F32)

        bfpool = ctx.enter_context(tc.tile_pool(name="bfstage", bufs=1))
        qstage = [bfpool.tile([P, C], BF16, name=f"q{mc}") for mc in range(NN)]
        kstage = [bfpool.tile([P, C], BF16, name=f"k{mc}") for mc in range(NN)]
        vstage = [bfpool.tile([P, C], BF16, name=f"v{mc}") for mc in range(NN)]

        # ---------------- prologue DMAs (plain loads only) ----------------
        # sync ring: q block 0, M, rest of q. scalar ring: k. gpsimd: v.
        for mc in range(4):
            nc.sync.dma_start(out=qstage[mc], in_=q_x[mc * P : (mc + 1) * P, :])
        for c1c in range(CC):
            nc.sync.dma_start(
                out=sbM8[:, c1c, :], in_=Mw[c1c * P : (c1c + 1) * P, :]
            )
        for mc in range(4, NN):
            nc.sync.dma_start(out=qstage[mc], in_=q_x[mc * P : (mc + 1) * P, :])
        for mc in range(NN):
            nc.scalar.dma_start(out=kstage[mc], in_=k_x[mc * P : (mc + 1) * P, :])

        # ---------------- PE warmup (HAM un-throttle) ----------------
        with (
            tc.tile_pool(name="warm", bufs=1) as warm_pool,
            tc.tile_pool(name="warm_psum", bufs=1, space="PSUM") as warm_psum,
        ):
            wl = warm_pool.tile([P, P], BF16)
            wr = warm_pool.tile([P, BLK], BF16)
            nc.vector.memset(wl, 0.0)
            nc.vector.memset(wr, 0.0)
            wps = warm_psum.tile([P, BLK], F32)
            for i in range(10):
                nc.tensor.matmul(wps, wl, wr, start=True, stop=True)

        make_identity(nc, identf)
        nc.vector.tensor_copy(out=identb, in_=identf)
        nc.vector.memset(all16, 16.0)
        nc.vector.memset(ln16, LN16)
        nc.vector.memset(onepad[:, :, 0:1], 1.0)
        nc.vector.memset(onepad[:, :, 1:], 0.0)
        nc.vector.tensor_copy(out=vb8[:, :, C:VW], in_=onepad)
        for mc in range(NN):
            nc.gpsimd.dma_start(out=vstage[mc], in_=v_x[mc * P : (mc + 1) * P, :])

        # ---------------- transpose + t phase ----------------
        t_psum = None

        def tr_group(nb, stage, dst, tag, pool=None, ptag="tr"):
                # PE-transpose 4 seq-tiles x 6 chunks into dst[:, cc, nb*BLK..]
                pool = pool or t_psum
                sl = slice(nb * BLK, (nb + 1) * BLK)
                for cc in range(CC):
                    trp = pool.tile(
                        [P, BLK], BF16, tag=ptag, name=f"{tag}{nb}_{cc}"
                    )
                    for j in range(4):
                        nc.tensor.transpose(
                            trp[:, j * P : (j + 1) * P],
                            stage[4 * nb + j][:, cc * P : (cc + 1) * P],
                            identb,
                        )
                    nc.vector.tensor_copy(out=dst[:, cc, sl], in_=trp)

        def tr_chunk(nb, cc, stage, dst, tag):
            sl = slice(nb * BLK, (nb + 1) * BLK)
            trp = t_psum.tile([P, BLK], BF16, tag="tr", name=f"{tag}{nb}_{cc}")
            for j in range(4):
                nc.tensor.transpose(
                    trp[:, j * P : (j + 1) * P],
                    stage[4 * nb + j][:, cc * P : (cc + 1) * P],
                    identb,
                )
            nc.vector.tensor_copy(out=dst[:, cc, sl], in_=trp)

        with tc.tile_pool(name="t_psum", bufs=4, space="PSUM") as t_psum:
            # transposes of block nb+1 (q then k) interleave between the six
            # t(nb) matmul groups: 12 tr-chunks per block, 2 per t-group
            tr_group(0, qstage, qT8, "trq")
            tr_group(0, kstage, kT8, "trk")
            for nb in range(NB):
                sl = slice(nb * BLK, (nb + 1) * BLK)
                for c2c in range(CC):
                    if nb + 1 < NB:
                        nxt = 2 * c2c
                        for x in (nxt, nxt + 1):
                            if x < CC:
                                tr_chunk(nb + 1, x, qstage, qT8, "trq")
                            else:
                                tr_chunk(nb + 1, x - CC, kstage, kT8, "trk")
                    tps = t_psum.tile([P, BLK], F32, tag="tp", name=f"t{nb}_{c2c}")
                    for i in range(CC // 2):
                        nc.tensor.matmul(
                            tps,
                            sbM8[:, 2 * i : 2 * i + 2, c2c * P : (c2c + 1) * P],
                            qT8[:, 2 * i : 2 * i + 2, sl],
                            start=(i == 0),
                            stop=(i == CC // 2 - 1),
                            perf_mode=DR,
                        )
                    nc.scalar.activation(out=tT8[:, c2c, sl], in_=tps, func=COPY)
            for mc in range(NN):
                nc.vector.tensor_copy(out=vb8[:, mc, 0:C], in_=vstage[mc])

        # ---------------- steady ----------------
        with (
            tc.tile_pool(name="pt_pool", bufs=2) as pt_pool,
            tc.tile_pool(name="e8_pool", bufs=2) as e8_pool,
            tc.tile_pool(name="out_pool", bufs=4) as out_pool,
            tc.tile_pool(name="sum_pool", bufs=4) as sum_pool,
            tc.tile_pool(name="rec_pool", bufs=4) as rec_pool,
            tc.tile_pool(name="s_psum", bufs=3, space="PSUM") as s_psum,
            tc.tile_pool(name="o_psum", bufs=2, space="PSUM") as o_psum,
            tc.tile_pool(name="o2_psum", bufs=2, space="PSUM") as o2_psum,
        ):
            PT16s = [
                pt_pool.tile([P, NN, BLK], BF16, tag="pt", name=f"pt{par}")
                for par in range(2)
            ]
            E8s = [
                e8_pool.tile([P, NN, BLK], F8, tag="e8", name=f"e8_{par}")
                for par in range(2)
            ]

            def s_block(nb):
                E8 = E8s[nb % 2]
                PT16 = PT16s[nb % 2]
                sl = slice(nb * BLK, (nb + 1) * BLK)
                for mc in range(NN):
                    sp = s_psum.tile([P, BLK], F32, tag="sp", name=f"sp{nb}_{mc}")
                    for i in range(CC // 2):
                        nc.tensor.matmul(
                            sp,
                            kT8[:, 2 * i : 2 * i + 2, mc * P : (mc + 1) * P],
                            tT8[:, 2 * i : 2 * i + 2, sl],
                            start=(i == 0),
                            stop=(i == CC // 2 - 1),
                            perf_mode=DR,
                        )
                    nc.scalar.activation(
                        out=PT16[:, mc, :], in_=sp, func=EXP,
                        scale=SCALE16, bias=ln16,
                    )
                    nc.vector.tensor_scalar(
                        out=E8[:, mc, :], in0=PT16[:, mc, :],
                        scalar1=16.0, scalar2=None, op0=SUB,
                    )

            def colsum_block():
                # 16*colsum(v) into idle PV psum slots; identical rows
                cs1 = o_psum.tile([P, BLK], F32, tag="op1", name="cs1")
                cs2 = o2_psum.tile([P, C - BLK], F32, tag="op2", name="cs2")
                for mc in range(NN):
                    nc.tensor.matmul(
                        cs1, all16, vstage[mc][:, 0:BLK],
                        start=(mc == 0), stop=(mc == NN - 1),
                    )
                    nc.tensor.matmul(
                        cs2, all16, vstage[mc][:, BLK:C],
                        start=(mc == 0), stop=(mc == NN - 1),
                    )
                nc.scalar.activation(out=csum[:, 0:BLK], in_=cs1, func=COPY)
                nc.scalar.activation(out=csum[:, BLK:C], in_=cs2, func=COPY)

            def pv_block(nb):
                E8 = E8s[nb % 2]
                for ns in range(4):
                    op1 = o_psum.tile([P, BLK], F32, tag="op1", name=f"o1_{nb}_{ns}")
                    op2 = o2_psum.tile(
                        [P, VW - BLK], F32, tag="op2", name=f"o2_{nb}_{ns}"
                    )
                    nsl = slice(ns * P, (ns + 1) * P)
                    for i in range(NN // 2):
                        lhs = E8[:, 2 * i : 2 * i + 2, nsl]
                        first = i == 0
                        last = i == NN // 2 - 1
                        nc.tensor.matmul(
                            op1, lhs, vb8[:, 2 * i : 2 * i + 2, 0:BLK],
                            start=first, stop=last,
                            perf_mode=DR,
                        )
                        nc.tensor.matmul(
                            op2, lhs, vb8[:, 2 * i : 2 * i + 2, BLK:VW],
                            start=first, stop=last,
                            perf_mode=DR,
                        )
                    # out = (O + 16*colsum) / (O_768 + 16*N): adds on DVE,
                    # reciprocal-scale on ACT, 16s cancel
                    zf = rec_pool.tile([P, 1], F32, tag="zf", name=f"zf{nb}_{ns}")
                    rec = rec_pool.tile([P, 1], F32, tag="rec", name=f"rc{nb}_{ns}")
                    nc.vector.tensor_scalar(
                        out=zf, in0=op2[:, C - BLK : C - BLK + 1],
                        scalar1=Z0, scalar2=None, op0=ADD,
                    )
                    nc.vector.reciprocal(out=rec, in_=zf)
                    t12 = sum_pool.tile(
                        [P, C], F32, tag="t12", name=f"t12_{nb}_{ns}"
                    )
                    nc.vector.tensor_tensor(
                        out=t12[:, 0:BLK], in0=op1, in1=csum[:, 0:BLK], op=ADD
                    )
                    nc.vector.tensor_tensor(
                        out=t12[:, BLK:C], in0=op2[:, 0 : C - BLK],
                        in1=csum[:, BLK:C], op=ADD,
                    )
                    o_t = out_pool.tile([P, C], BF16, tag="ot", name=f"ot{nb}_{ns}")
                    nc.scalar.activation(
                        out=o_t[:, 0:BLK], in_=t12[:, 0:BLK], func=COPY, scale=rec
                    )
                    nc.scalar.activation(
                        out=o_t[:, BLK:C], in_=t12[:, BLK:C], func=COPY, scale=rec
                    )
                    row0 = nb * BLK + ns * P
                    if nb == NB - 1:
                        nc.sync.dma_start(
                            out=out[row0 : row0 + P, 0 : C // 2],
                            in_=o_t[:, 0 : C // 2],
                        )
                        nc.scalar.dma_start(
                            out=out[row0 : row0 + P, C // 2 : C],
                            in_=o_t[:, C // 2 : C],
                        )
                    else:
                        ring = nc.sync if ns % 2 == 0 else nc.gpsimd
                        ring.dma_start(out=out[row0 : row0 + P, :], in_=o_t)

            colsum_block()
            s_block(0)
            s_block(1)
            pv_block(0)
            s_block(2)
            pv_block(1)
            s_block(3)
            pv_block(2)
            pv_block(3)

    nc.compile()
    return nc


_NC = None


def _get_nc():
    global _NC
    if _NC is None:
        _NC = build_kernel()
    return _NC


def kernel(q_x, k_x, v_x, Wq, Wk):
    import ml_dtypes
    from concourse.bass_utils import run_bass_kernel_spmd

    bf = ml_dtypes.bfloat16
    f8 = ml_dtypes.float8_e4m3
    q_x = np.ascontiguousarray(np.asarray(q_x, dtype=np.float32)).astype(bf)
    k_x = np.ascontiguousarray(np.asarray(k_x, dtype=np.float32)).astype(bf)
    v_x = np.ascontiguousarray(np.asarray(v_x, dtype=np.float32)).astype(bf)
    Wq = np.asarray(Wq, dtype=np.float32)
    Wk = np.asarray(Wk, dtype=np.float32)
    # weight folding: S = q_x (Wq^T Wk) k_x^T; x16 to center fp8 range
    Mw = np.ascontiguousarray(16.0 * (Wq.T @ Wk)).astype(f8)

    nc = _get_nc()
    in_maps = [
        {"q_x": q_x[i], "k_x": k_x[i], "v_x": v_x[i], "Mw": Mw}
        for i in range(B)
    ]
    res = run_bass_kernel_spmd(nc, in_maps, core_ids=list(range(B)))
    return np.stack(
        [res.results[i]["out"].astype(np.float32) for i in range(B)], axis=0
    )
